# revision 1
# baseline (speedup 1.0000x reference)
"""Distributed causal attention (RoPE) kernel for 8 TRN2 NeuronCores.

Problem: B=4, S=2048, dim=2048, H=16 heads, D=128 head dim.
  q,k,v = x @ W{q,k,v}.T (heads), RoPE(q,k), causal softmax(q k^T/sqrt(D)) v,
  out = concat_heads @ Wo.T

Sharding: tensor-parallel over heads — 2 heads per core. Each core:
  - computes qT/kT [d, t] and v [s, e] for its 2 heads (weights pre-transposed
    host-side so every matmul operand is in its natural layout),
  - attention in "scoresT" orientation [key s on partitions, query t free]:
    exp without max-subtraction (bf16 holds e^16 fine); the softmax
    denominator comes from accumulating exp tiles on DVE (bf16 adds) and ONE
    all-ones [128,128] stationary matmul per query tile — 10x fewer TensorE
    columns than a ones-matmul per key tile,
  - per-batch All-to-All (both heads packed) reshards attention output from
    head-shard to row-shard; out-projection for batch b runs after A2A(b+1)
    is issued so every collective hides behind compute,
  - row-local output projection; host reassembles the row shards.

Pipelined per batch: P1(b) proj -> P2(b,h) attention -> A2A(b) -> P4(b-1)
out-proj, keeping TensorE dense while collectives/DMAs fly.
"""

import numpy as np
import ml_dtypes

B, S, DIM = 4, 2048, 2048
H, D = 16, 128
NCORES = 8
HPC = H // NCORES            # heads per core = 2
E = HPC * D                  # per-core inner width = 256
BS = B * S                   # 8192 flattened rows
KT = DIM // 128              # 16 contraction tiles
TQ = 512                     # query tile width
NQ = S // TQ                 # 4 query tiles per (b,h)
NB = S // TQ                 # 4 x-blocks per batch
RPB = S // NCORES            # 256 output rows per core per batch
ROWS = B * RPB               # 1024 output rows per core
SCALE = 1.0 / np.sqrt(D)

_CACHE = {}


def _build(causal: bool):
    from concourse import bacc, tile, mybir

    f32 = mybir.dt.float32
    bf16 = mybir.dt.bfloat16
    Exp = mybir.ActivationFunctionType.Exp

    nc = bacc.Bacc(None, target_bir_lowering=False, num_devices=NCORES)

    # host layouts: xT pre-tiled [block n, ktile, 128, 512]
    xT_d = nc.dram_tensor("xT", [B * NB, KT, 128, TQ], bf16, kind="ExternalInput")
    wq_d = nc.dram_tensor("wqT", [KT, 128, E], bf16, kind="ExternalInput")
    wk_d = nc.dram_tensor("wkT", [KT, 128, E], bf16, kind="ExternalInput")
    wv_d = nc.dram_tensor("wvT", [KT, 128, E], bf16, kind="ExternalInput")
    wo_d = nc.dram_tensor("woT", [DIM // TQ, KT, 128, TQ], bf16, kind="ExternalInput")
    cos_d = nc.dram_tensor("cosT", [128, BS], bf16, kind="ExternalInput")
    sin_d = nc.dram_tensor("sinT", [128, BS], bf16, kind="ExternalInput")
    msk_d = nc.dram_tensor("masks", [4, 128, TQ], bf16, kind="ExternalInput")
    out_d = nc.dram_tensor("out", [ROWS, DIM], f32, kind="ExternalOutput")

    with tile.TileContext(nc) as tc:
        with (
            tc.tile_pool(name="const", bufs=1) as constp,
            tc.tile_pool(name="dram", bufs=1, space="DRAM") as dramp,
        ):
            a2a_in = [[dramp.tile([NCORES, 128, RPB], bf16, name=f"a2ai{b}h{h}")
                       for h in range(HPC)] for b in range(B)]
            a2a_out = [[dramp.tile([NCORES, 128, RPB], bf16, name=f"a2ao{b}h{h}")
                        for h in range(HPC)] for b in range(B)]

            ones_col = constp.tile([128, 128], bf16)
            nc.gpsimd.memset(ones_col[:], 1.0)

            # startup DMAs, finest-first so the first projection matmul can
            # begin after ~2 chunks: wq chunks stream on the scalar ring while
            # x-block 0 streams per-ktile on sync/gpsimd.
            wq_sb = constp.tile([128, KT, E], bf16)
            wk_sb = constp.tile([128, KT, E], bf16)
            wv_sb = constp.tile([128, KT, E], bf16)
            for k in range(0, KT, 2):
                nc.scalar.dma_start(wq_sb[:, k:k + 2, :],
                                    wq_d[k:k + 2].rearrange("k p e -> p k e"))
            for k in range(0, KT, 2):
                nc.scalar.dma_start(wk_sb[:, k:k + 2, :],
                                    wk_d[k:k + 2].rearrange("k p e -> p k e"))
            for k in range(0, KT, 2):
                nc.scalar.dma_start(wv_sb[:, k:k + 2, :],
                                    wv_d[k:k + 2].rearrange("k p e -> p k e"))
            if causal:
                msk_sb = constp.tile([128, 4, TQ], bf16)
                for o in range(4):
                    nc.gpsimd.dma_start(msk_sb[:, o, :], msk_d[o])

            with (
                tc.tile_pool(name="qkv", bufs=2) as qkvp,
                tc.tile_pool(name="xblk", bufs=2) as xp,
                tc.tile_pool(name="cs", bufs=2) as cp,
                tc.tile_pool(name="rope", bufs=2) as rp,
                tc.tile_pool(name="att", bufs=4) as ap,
                tc.tile_pool(name="ex", bufs=6) as exp_pool,
                tc.tile_pool(name="exa", bufs=2) as exap,
                tc.tile_pool(name="wo", bufs=2) as wop,
                tc.tile_pool(name="attin", bufs=2) as atp,
                tc.tile_pool(name="res", bufs=3) as resp,
                tc.tile_pool(name="ps1", bufs=1, space="PSUM") as pp1,
                tc.tile_pool(name="ps2", bufs=1, space="PSUM") as pp2,
            ):

                def attention(b, h, qb, kb, vb):
                    """Attention for (batch b, local head h) -> a2a_in[b][h].

                    Returns the last ot tile (used as a scheduling token: the
                    Tile scheduler's sim under-estimates collective latency,
                    so downstream out-proj work is data-bound to attention
                    completion to keep it from stalling the tensor queue)."""
                    ots = []
                    for tq in range(NQ):
                        t0 = tq * TQ
                        jmax = (tq + 1) * (TQ // 128) if causal else S // 128
                        av = pp2.tile([128, TQ], f32, tag="av", bufs=2)
                        exa = exap.tile([128, TQ], bf16, tag="exa", bufs=2)
                        for j in range(jmax):
                            s0 = j * 128
                            # diagonal tiles: only queries t >= s attend;
                            # compute the trapezoid [off:TQ) at reduced width
                            diag = causal and j >= jmax - 4
                            off = 128 * (j - (jmax - 4)) if diag else 0
                            w = TQ - off
                            sc = pp2.tile([128, TQ], f32, tag="sc", bufs=2)
                            nc.tensor.matmul(
                                sc[:, 0:w], kb[:, h, s0:s0 + 128],
                                qb[:, h, t0 + off:t0 + TQ],
                                start=True, stop=True,
                            )
                            ex = exp_pool.tile([128, TQ], bf16, tag="ex")
                            nc.scalar.activation(ex[:, 0:w], sc[:, 0:w], Exp,
                                                 scale=float(SCALE))
                            if diag:
                                exm = exp_pool.tile([128, TQ], bf16, tag="exm",
                                                    bufs=2)
                                nc.vector.tensor_mul(exm[:, 0:w], ex[:, 0:w],
                                                     msk_sb[:, 0, 0:w])
                                ex = exm
                            nc.tensor.matmul(
                                av[:, off:TQ], vb[:, j, h * 128:(h + 1) * 128],
                                ex[:, 0:w],
                                start=(j == 0), stop=(j == jmax - 1),
                            )
                            # softmax denominator: accumulate exp tiles in
                            # bf16 on DVE; one ones-matmul per query tile
                            if j == 0:
                                nc.vector.tensor_copy(exa[:], ex[:])
                            else:
                                nc.vector.tensor_add(exa[:, off:TQ],
                                                     exa[:, off:TQ],
                                                     ex[:, 0:w])
                        cs = pp2.tile([128, TQ], f32, tag="cs", bufs=1)
                        nc.tensor.matmul(cs[:], ones_col[:], exa[:],
                                         start=True, stop=True)
                        # 1/colsum: approx reciprocal (~18 bits, 1 DVE op)
                        rec = ap.tile([128, TQ], f32, tag="rec", bufs=2)
                        nc.vector.reciprocal_approx_fast(rec[:], cs[:])
                        ot = ap.tile([128, TQ], bf16, tag="ot", bufs=3)
                        nc.vector.tensor_mul(ot[:], av[:], rec[:])
                        # queries [t0, t0+512) of batch b go to dest cores
                        # 2tq (first 256) and 2tq+1 (second 256)
                        nc.sync.dma_start(a2a_in[b][h][2 * tq], ot[:, 0:RPB])
                        nc.sync.dma_start(a2a_in[b][h][2 * tq + 1],
                                          ot[:, RPB:2 * RPB])
                        ots.append(ot)
                    return ots

                def out_proj(b, dep_ots):
                    """Out-projection for my RPB rows of batch b; at_sb loads
                    are token-bound to dep_ots[1] (mid-attention) so the
                    scheduler cannot place this work where the collective is
                    still in flight, while loads prefetch during the
                    attention tail."""
                    at_sb = atp.tile([128, KT, RPB], bf16, tag="at")
                    nc.vector.tensor_copy(at_sb[0:1, :, 0:1],
                                          dep_ots[1][0:1, 0:KT])
                    for i in range(NCORES):
                        eng = nc.sync if i % 2 == 0 else nc.scalar
                        eng.dma_start(at_sb[:, 2 * i, :], a2a_out[b][0][i])
                        eng.dma_start(at_sb[:, 2 * i + 1, :],
                                      a2a_out[b][1][i])
                    for f in range(DIM // TQ):
                        wo_f = wop.tile([128, KT, TQ], bf16, tag="wo", bufs=2)
                        nc.scalar.dma_start(wo_f[:],
                                            wo_d[f].rearrange("k p t -> p k t"))
                        for tt in range(RPB // 128):
                            ops = pp1.tile([128, TQ], f32, tag="qk", bufs=2)
                            for ki in range(KT):
                                nc.tensor.matmul(
                                    ops[:],
                                    at_sb[:, ki, tt * 128:(tt + 1) * 128],
                                    wo_f[:, ki, :],
                                    start=(ki == 0), stop=(ki == KT - 1),
                                )
                            res = resp.tile([128, TQ], f32, tag="res")
                            nc.vector.tensor_copy(res[:], ops[:])
                            oeng = nc.sync if tt % 2 == 0 else nc.gpsimd
                            oeng.dma_start(
                                out_d[b * RPB + tt * 128:
                                      b * RPB + (tt + 1) * 128,
                                      f * TQ:(f + 1) * TQ],
                                res[:])

                def tail_projections(dep_ots):
                    """Fused tail: P4(B-2) full-contraction and P4(B-1)
                    pass A (head-0 k-tiles, data from A2A(B-1,0)) share each
                    wo tile f-major — ~53us of TensorE cover over the last
                    collective's peer rendezvous with half the wo traffic.
                    Pass B runs once A2A(B-1,1) lands."""
                    b2, b3 = B - 2, B - 1
                    at2 = atp.tile([128, KT, RPB], bf16, tag="at")
                    nc.vector.tensor_copy(at2[0:1, :, 0:1],
                                          dep_ots[1][0:1, 0:KT])
                    for i in range(NCORES):
                        eng = nc.sync if i % 2 == 0 else nc.scalar
                        eng.dma_start(at2[:, 2 * i, :], a2a_out[b2][0][i])
                        eng.dma_start(at2[:, 2 * i + 1, :], a2a_out[b2][1][i])
                    # first two wo tiles stream ahead of the collective-gated
                    # at3 loads so the fused loop is never ring-blocked
                    wo_tiles = {}
                    for f in (0, 1):
                        wo_f = wop.tile([128, KT, TQ], bf16, tag="wo", bufs=2)
                        weng = nc.scalar if f % 2 == 0 else nc.sync
                        weng.dma_start(wo_f[:],
                                       wo_d[f].rearrange("k p t -> p k t"))
                        wo_tiles[f] = wo_f
                    at3 = atp.tile([128, KT, RPB], bf16, tag="at")
                    nc.vector.tensor_copy(at3[0:1, :, 0:1],
                                          dep_ots[1][0:1, 0:KT])
                    for i in range(NCORES):
                        eng = nc.scalar if i % 2 == 0 else nc.sync
                        eng.dma_start(at3[:, 2 * i, :], a2a_out[b3][0][i])
                    partials = {}

                    def at2_group(f, wo_f):
                        for tt in range(RPB // 128):
                            ops = pp1.tile([128, TQ], f32, tag="qk", bufs=2)
                            for ki in range(KT):
                                nc.tensor.matmul(
                                    ops[:],
                                    at2[:, ki, tt * 128:(tt + 1) * 128],
                                    wo_f[:, ki, :],
                                    start=(ki == 0), stop=(ki == KT - 1),
                                )
                            res = resp.tile([128, TQ], f32, tag="res")
                            nc.vector.tensor_copy(res[:], ops[:])
                            oeng = nc.sync if tt % 2 == 0 else nc.gpsimd
                            oeng.dma_start(
                                out_d[b2 * RPB + tt * 128:
                                      b2 * RPB + (tt + 1) * 128,
                                      f * TQ:(f + 1) * TQ],
                                res[:])

                    def passA_group(f, wo_f):
                        for tt in range(RPB // 128):
                            opsA = pp1.tile([128, TQ], f32, tag="qk", bufs=2)
                            for i in range(NCORES):
                                nc.tensor.matmul(
                                    opsA[:],
                                    at3[:, 2 * i, tt * 128:(tt + 1) * 128],
                                    wo_f[:, 2 * i, :],
                                    start=(i == 0), stop=(i == NCORES - 1),
                                )
                            pa = resp.tile([128, TQ], bf16, tag="pa",
                                           name=f"pa{f}_{tt}", bufs=8)
                            nc.vector.tensor_copy(pa[:], opsA[:])
                            partials[(f, tt)] = pa

                    # front-load at2 (data long ready) so the in-order tensor
                    # queue never stalls on pass A's collective; passA(f) must
                    # precede the wo alloc of f+2 (2-buf ring)
                    for kind, f in (("at2", 0), ("at2", 1), ("pA", 0),
                                    ("at2", 2), ("pA", 1), ("at2", 3),
                                    ("pA", 2), ("pA", 3)):
                        if f not in wo_tiles:
                            wo_f = wop.tile([128, KT, TQ], bf16, tag="wo",
                                            bufs=2)
                            weng = nc.scalar if f % 2 == 0 else nc.sync
                            weng.dma_start(
                                wo_f[:], wo_d[f].rearrange("k p t -> p k t"))
                            wo_tiles[f] = wo_f
                        if kind == "at2":
                            at2_group(f, wo_tiles[f])
                        else:
                            passA_group(f, wo_tiles[f])
                    for i in range(NCORES):
                        eng = nc.sync if i % 2 == 0 else nc.scalar
                        eng.dma_start(at3[:, 2 * i + 1, :], a2a_out[b3][1][i])
                    # pass B: f=2,3 reuse resident wo tiles; f=0,1 re-stream
                    # behind them (2,3,0,1 order keeps re-streams off the
                    # critical path)
                    for f in (2, 3, 0, 1):
                        if f >= 2:
                            wo_f = wo_tiles[f]
                        else:
                            wo_f = wop.tile([128, KT, TQ], bf16, tag="wo",
                                            bufs=2)
                            weng = nc.scalar if f % 2 == 0 else nc.sync
                            weng.dma_start(
                                wo_f[:], wo_d[f].rearrange("k p t -> p k t"))
                        for tt in range(RPB // 128):
                            ops = pp1.tile([128, TQ], f32, tag="qk", bufs=2)
                            for i in range(NCORES):
                                nc.tensor.matmul(
                                    ops[:],
                                    at3[:, 2 * i + 1, tt * 128:(tt + 1) * 128],
                                    wo_f[:, 2 * i + 1, :],
                                    start=(i == 0), stop=(i == NCORES - 1),
                                )
                            res = resp.tile([128, TQ], f32, tag="res")
                            nc.vector.tensor_add(
                                res[:], ops[:], partials[(f, tt)][:])
                            oeng = nc.sync if tt % 2 == 0 else nc.gpsimd
                            oeng.dma_start(
                                out_d[b3 * RPB + tt * 128:
                                      b3 * RPB + (tt + 1) * 128,
                                      f * TQ:(f + 1) * TQ],
                                res[:])

                for b in range(B):
                    # ---- P1(b): projections + RoPE ----
                    qb = qkvp.tile([128, HPC, S], bf16, tag="q", name=f"q{b}")
                    kb = qkvp.tile([128, HPC, S], bf16, tag="k", name=f"k{b}")
                    vb = qkvp.tile([128, S // 128, E], bf16, tag="v", name=f"v{b}")
                    for n in range(NB):
                        c0 = n * TQ
                        g0 = b * S + c0
                        xblk = xp.tile([128, KT, TQ], bf16, tag="xblk")
                        cos_b = cp.tile([128, TQ], bf16, tag="cos")
                        sin_b = cp.tile([128, TQ], bf16, tag="sin")
                        if b == 0 and n == 0:
                            # finest-grained first load + cos/sin ahead of
                            # the x tiles on the sync ring
                            nc.sync.dma_start(cos_b[:], cos_d[:, g0:g0 + TQ])
                            nc.sync.dma_start(sin_b[:], sin_d[:, g0:g0 + TQ])
                            for k in range(KT):
                                eng = (nc.sync if k < 4 or k % 2 == 0
                                       else nc.gpsimd)
                                eng.dma_start(xblk[:, k, :], xT_d[0, k])
                        else:
                            nc.scalar.dma_start(cos_b[:], cos_d[:, g0:g0 + TQ])
                            nc.scalar.dma_start(sin_b[:], sin_d[:, g0:g0 + TQ])
                            for k in range(0, KT, 4):
                                eng = nc.sync if (k // 4) % 2 == 0 else nc.gpsimd
                                eng.dma_start(
                                    xblk[:, k:k + 4, :],
                                    xT_d[b * NB + n, k:k + 4].rearrange(
                                        "k p t -> p k t"))

                        for w_sb, dst in ((wq_sb, qb), (wk_sb, kb)):
                            for h in range(HPC):
                                ps = pp1.tile([128, TQ], f32, tag="qk", bufs=2)
                                for k in range(KT):
                                    nc.tensor.matmul(
                                        ps[:], w_sb[:, k, h * 128:(h + 1) * 128],
                                        xblk[:, k, :],
                                        start=(k == 0), stop=(k == KT - 1),
                                    )
                                t0_ = rp.tile([128, TQ], f32, tag="t0", bufs=2)
                                nc.vector.tensor_mul(t0_[0:64, :], ps[64:128, :], sin_b[0:64, :])
                                nc.vector.tensor_mul(t0_[64:128, :], ps[0:64, :], sin_b[64:128, :])
                                t1_ = rp.tile([128, TQ], f32, tag="t1", bufs=2)
                                nc.vector.tensor_mul(t1_[:], ps[:], cos_b[:])
                                nc.vector.tensor_add(dst[:, h, c0:c0 + TQ], t0_[:], t1_[:])

                        for ss in range(TQ // 128):
                            vps = pp1.tile([128, E], f32, tag="v", bufs=1)
                            for k in range(KT):
                                nc.tensor.matmul(
                                    vps[:], xblk[:, k, ss * 128:(ss + 1) * 128],
                                    wv_sb[:, k, :],
                                    start=(k == 0), stop=(k == KT - 1),
                                )
                            nc.vector.tensor_copy(vb[:, n * 4 + ss, :], vps[:])

                    # ---- P2(b): attention; per-head A2A so rendezvous
                    # starts early and the tail can split passes ----
                    ot0 = attention(b, 0, qb, kb, vb)
                    nc.gpsimd.collective_compute(
                        "AllToAll", mybir.AluOpType.bypass,
                        replica_groups=[list(range(NCORES))],
                        ins=[a2a_in[b][0][:].opt()],
                        outs=[a2a_out[b][0][:].opt()],
                    )
                    # P4(b-1) interleaves with P2(b,1); its collectives
                    # finished a batch ago (token keeps the scheduler from
                    # placing it any earlier).
                    if 1 <= b < B - 1:
                        out_proj(b - 1, ot0)
                    ot1 = attention(b, 1, qb, kb, vb)
                    nc.gpsimd.collective_compute(
                        "AllToAll", mybir.AluOpType.bypass,
                        replica_groups=[list(range(NCORES))],
                        ins=[a2a_in[b][1][:].opt()],
                        outs=[a2a_out[b][1][:].opt()],
                    )

                tail_projections(ot1)

    nc.compile()
    return nc


def _prep_inputs(x, Wq, Wk, Wv, Wo, causal):
    bf16 = ml_dtypes.bfloat16
    xT = np.ascontiguousarray(x.reshape(BS, DIM).T).astype(bf16)  # [dim, BS]
    # pre-tile: [block n, ktile, 128, 512]
    xTt = np.ascontiguousarray(
        xT.reshape(KT, 128, B * NB, TQ).transpose(2, 0, 1, 3))
    woT = np.ascontiguousarray(Wo.T).astype(bf16)                 # [e, f]
    woTt = np.ascontiguousarray(
        woT.reshape(KT, 128, DIM // TQ, TQ).transpose(2, 0, 1, 3))

    # RoPE tables in [d, pos] layout, tiled over batches; sin pre-signed for
    # rotate_half (rows 0:64 multiply the shifted-up half, hence negative).
    inv_freq = 1.0 / (10000.0 ** (np.arange(0, D, 2, dtype=np.float64) / D))
    t = np.arange(S, dtype=np.float64)
    freqs = np.outer(t, inv_freq)                      # [S, 64]
    emb = np.concatenate([freqs, freqs], axis=-1)      # [S, D]
    cosT = np.tile(np.cos(emb).T.astype(np.float32), (1, B)).astype(bf16)
    sinN = np.sin(emb).T.astype(np.float32)
    sinN[0:64] *= -1.0
    sinT = np.tile(sinN, (1, B)).astype(bf16)

    masks = np.zeros((4, 128, TQ), dtype=bf16)
    ii = np.arange(128)[:, None]
    jj = np.arange(TQ)[None, :]
    for o in range(4):
        masks[o] = (jj >= ii + 128 * o).astype(bf16)

    in_maps = []
    for c in range(NCORES):
        e0, e1 = c * E, (c + 1) * E
        in_maps.append({
            "xT": xTt,
            "wqT": np.ascontiguousarray(Wq[e0:e1].T).astype(bf16).reshape(KT, 128, E),
            "wkT": np.ascontiguousarray(Wk[e0:e1].T).astype(bf16).reshape(KT, 128, E),
            "wvT": np.ascontiguousarray(Wv[e0:e1].T).astype(bf16).reshape(KT, 128, E),
            "woT": woTt,
            "cosT": cosT,
            "sinT": sinT,
            "masks": masks,
        })
    return in_maps


def kernel(x, Wq, Wk, Wv, Wo, mask, _trace=False):
    from concourse.bass_utils import run_bass_kernel_spmd

    m = np.asarray(mask)
    causal = not bool(m.reshape(m.shape[-2], m.shape[-1])[0, -1])

    if causal not in _CACHE:
        _CACHE[causal] = _build(causal)
    nc = _CACHE[causal]

    in_maps = _prep_inputs(np.asarray(x), np.asarray(Wq), np.asarray(Wk),
                           np.asarray(Wv), np.asarray(Wo), causal)
    res = run_bass_kernel_spmd(nc, in_maps, core_ids=list(range(NCORES)),
                               trace=_trace)
    # core c holds rows [c*RPB, (c+1)*RPB) of every batch, b-major
    full = np.empty((B, S, DIM), np.float32)
    for c in range(NCORES):
        rc = res.results[c]["out"].reshape(B, RPB, DIM)
        full[:, c * RPB:(c + 1) * RPB, :] = rc.astype(np.float32)
    if _trace:
        return full, res
    return full



# revision 2
# speedup vs baseline: 1.1217x; 1.1217x over previous
"""Distributed causal attention (RoPE) kernel for 8 TRN2 NeuronCores.

Problem: B=4, S=2048, dim=2048, H=16 heads, D=128 head dim.
  q,k,v = x @ W{q,k,v}.T (heads), RoPE(q,k), causal softmax(q k^T/sqrt(D)) v,
  out = concat_heads @ Wo.T

Sharding: tensor-parallel over heads — 2 heads per core. Each core:
  - computes qT/kT [d, t] and v [s, e] for its 2 heads (weights pre-transposed
    host-side so every matmul operand is in its natural layout),
  - attention in "scoresT" orientation [key s on partitions, query t free]:
    exp without max-subtraction (bf16 holds e^16 fine); the softmax
    denominator comes from accumulating exp tiles on DVE (bf16 adds) and ONE
    all-ones [128,128] stationary matmul per query tile,
  - per-(batch,head) All-to-All (DRAM->DRAM) reshards attention output from
    head-shard to row-shard,
  - row-local output projection; host reassembles the row shards.

Schedule (v2): PE streams at ~0.5ns/col on this part, so the only wins are
scheduling. Warmup matmuls flip the HAM clock gate before real work arrives.
Out-projections for batches 0,1,3 are DEFERRED to an f-major tail (~100us of
matmul) that covers the last two collectives; only out_proj(b2) stays
interleaved with batch-3 attention as TensorE filler for the ACT-bound exp
chain. The ACT queue carries nothing but exps steady-state (weight/wo/at DMA
triggers live on scalar only at start/tail; x/cos/sin/ot on sync; collectives
and output stores on gpsimd). Output is written bf16 (err budget allows) to
halve store traffic.
"""

import numpy as np
import ml_dtypes

B, S, DIM = 4, 2048, 2048
H, D = 16, 128
NCORES = 8
HPC = H // NCORES            # heads per core = 2
E = HPC * D                  # per-core inner width = 256
BS = B * S                   # 8192 flattened rows
KT = DIM // 128              # 16 contraction tiles
TQ = 512                     # query tile width
NQ = S // TQ                 # 4 query tiles per (b,h)
NB = S // TQ                 # 4 x-blocks per batch
RPB = S // NCORES            # 256 output rows per core per batch
ROWS = B * RPB               # 1024 output rows per core
SCALE = 1.0 / np.sqrt(D)
WARM = 34                    # HAM warmup matmuls (~10us at mixed clock)

_CACHE = {}


def _build(causal: bool):
    from concourse import bacc, tile, mybir

    f32 = mybir.dt.float32
    bf16 = mybir.dt.bfloat16
    Exp = mybir.ActivationFunctionType.Exp

    nc = bacc.Bacc(None, target_bir_lowering=False, num_devices=NCORES)

    # host layouts: xT pre-tiled [block n, ktile, 128, 512]
    xT_d = nc.dram_tensor("xT", [B * NB, KT, 128, TQ], bf16, kind="ExternalInput")
    wq_d = nc.dram_tensor("wqT", [KT, 128, E], bf16, kind="ExternalInput")
    wk_d = nc.dram_tensor("wkT", [KT, 128, E], bf16, kind="ExternalInput")
    wv_d = nc.dram_tensor("wvT", [KT, 128, E], bf16, kind="ExternalInput")
    wo_d = nc.dram_tensor("woT", [DIM // TQ, KT, 128, TQ], bf16, kind="ExternalInput")
    cos_d = nc.dram_tensor("cosT", [128, BS], bf16, kind="ExternalInput")
    sin_d = nc.dram_tensor("sinT", [128, BS], bf16, kind="ExternalInput")
    msk_d = nc.dram_tensor("masks", [128, TQ], bf16, kind="ExternalInput")
    out_d = nc.dram_tensor("out", [ROWS, DIM], bf16, kind="ExternalOutput")

    with tile.TileContext(nc) as tc:
        with (
            tc.tile_pool(name="const", bufs=1) as constp,
            tc.tile_pool(name="dram", bufs=1, space="DRAM") as dramp,
        ):
            a2a_in = [[dramp.tile([NCORES, 128, RPB], bf16, name=f"a2ai{b}h{h}")
                       for h in range(HPC)] for b in range(B)]
            a2a_out = [[dramp.tile([NCORES, 128, RPB], bf16, name=f"a2ao{b}h{h}")
                        for h in range(HPC)] for b in range(B)]

            ones_col = constp.tile([128, 128], bf16)
            nc.gpsimd.memset(ones_col[:], 1.0)
            warm_x = constp.tile([128, TQ], bf16)
            nc.gpsimd.memset(warm_x[:], 0.0)

            # startup DMAs on the scalar ring (free until first exp), finest
            # first so the first projection matmul can begin after ~2 chunks.
            wq_sb = constp.tile([128, KT, E], bf16)
            wk_sb = constp.tile([128, KT, E], bf16)
            wv_sb = constp.tile([128, KT, E], bf16)
            for k in range(0, KT, 2):
                nc.scalar.dma_start(wq_sb[:, k:k + 2, :],
                                    wq_d[k:k + 2].rearrange("k p e -> p k e"))
            for k in range(0, KT, 2):
                nc.scalar.dma_start(wk_sb[:, k:k + 2, :],
                                    wk_d[k:k + 2].rearrange("k p e -> p k e"))
            for k in range(0, KT, 2):
                nc.scalar.dma_start(wv_sb[:, k:k + 2, :],
                                    wv_d[k:k + 2].rearrange("k p e -> p k e"))
            if causal:
                msk_sb = constp.tile([128, TQ], bf16)
                nc.gpsimd.dma_start(msk_sb[:], msk_d[:])

            with (
                tc.tile_pool(name="qkv", bufs=2) as qkvp,
                tc.tile_pool(name="xblk", bufs=2) as xp,
                tc.tile_pool(name="cs", bufs=2) as cp,
                tc.tile_pool(name="rope", bufs=2) as rp,
                tc.tile_pool(name="att", bufs=4) as ap,
                tc.tile_pool(name="ex", bufs=6) as exp_pool,
                tc.tile_pool(name="exa", bufs=2) as exap,
                tc.tile_pool(name="wo", bufs=2) as wop,
                tc.tile_pool(name="attin", bufs=4) as atp,
                tc.tile_pool(name="res", bufs=4) as resp,
                tc.tile_pool(name="ps1", bufs=1, space="PSUM") as pp1,
                tc.tile_pool(name="ps2", bufs=1, space="PSUM") as pp2,
            ):
                # HAM warmup: garbage matmuls (zeros) so the clock gate is at
                # 8/8 by the time the first data-dependent matmul issues.
                for _ in range(WARM):
                    wps = pp1.tile([128, TQ], f32, tag="qk", bufs=2)
                    nc.tensor.matmul(wps[:], ones_col[:], warm_x[:],
                                     start=True, stop=True)

                def p1(b):
                    """Projections + RoPE for batch b -> qb, kb, vb."""
                    qb = qkvp.tile([128, HPC, S], bf16, tag="q", name=f"q{b}")
                    kb = qkvp.tile([128, HPC, S], bf16, tag="k", name=f"k{b}")
                    vb = qkvp.tile([128, S // 128, E], bf16, tag="v",
                                   name=f"v{b}")
                    for n in range(NB):
                        c0 = n * TQ
                        g0 = b * S + c0
                        xblk = xp.tile([128, KT, TQ], bf16, tag="xblk")
                        cos_b = cp.tile([128, TQ], bf16, tag="cos")
                        sin_b = cp.tile([128, TQ], bf16, tag="sin")
                        nc.sync.dma_start(cos_b[:], cos_d[:, g0:g0 + TQ])
                        nc.sync.dma_start(sin_b[:], sin_d[:, g0:g0 + TQ])
                        if b == 0 and n == 0:
                            # finest-grained first load so matmuls start ASAP
                            for k in range(KT):
                                nc.sync.dma_start(xblk[:, k, :], xT_d[0, k])
                        else:
                            for k in range(0, KT, 8):
                                nc.sync.dma_start(
                                    xblk[:, k:k + 8, :],
                                    xT_d[b * NB + n, k:k + 8].rearrange(
                                        "k p t -> p k t"))

                        for w_sb, dst in ((wq_sb, qb), (wk_sb, kb)):
                            for h in range(HPC):
                                ps = pp1.tile([128, TQ], f32, tag="qk", bufs=2)
                                for k in range(KT):
                                    nc.tensor.matmul(
                                        ps[:],
                                        w_sb[:, k, h * 128:(h + 1) * 128],
                                        xblk[:, k, :],
                                        start=(k == 0), stop=(k == KT - 1),
                                    )
                                t0_ = rp.tile([128, TQ], f32, tag="t0", bufs=2)
                                nc.vector.tensor_mul(t0_[0:64, :],
                                                     ps[64:128, :],
                                                     sin_b[0:64, :])
                                nc.vector.tensor_mul(t0_[64:128, :],
                                                     ps[0:64, :],
                                                     sin_b[64:128, :])
                                t1_ = rp.tile([128, TQ], f32, tag="t1", bufs=2)
                                nc.vector.tensor_mul(t1_[:], ps[:], cos_b[:])
                                nc.vector.tensor_add(dst[:, h, c0:c0 + TQ],
                                                     t0_[:], t1_[:])

                        for ss in range(TQ // 128):
                            vps = pp1.tile([128, E], f32, tag="v", bufs=1)
                            for k in range(KT):
                                nc.tensor.matmul(
                                    vps[:],
                                    xblk[:, k, ss * 128:(ss + 1) * 128],
                                    wv_sb[:, k, :],
                                    start=(k == 0), stop=(k == KT - 1),
                                )
                            nc.vector.tensor_copy(vb[:, n * 4 + ss, :], vps[:])
                    return qb, kb, vb

                def attention(b, h, qb, kb, vb):
                    """Attention for (batch b, local head h) -> a2a_in[b][h].
                    Returns the per-qtile ot tiles (used as scheduling
                    tokens)."""
                    ots = []
                    for tq in range(NQ):
                        t0 = tq * TQ
                        jmax = (tq + 1) * (TQ // 128) if causal else S // 128
                        av = pp2.tile([128, TQ], f32, tag="av", bufs=2)
                        exa = exap.tile([128, TQ], bf16, tag="exa", bufs=2)
                        for j in range(jmax):
                            s0 = j * 128
                            # diagonal tiles: only queries t >= s attend;
                            # compute the trapezoid [off:TQ) at reduced width
                            diag = causal and j >= jmax - 4
                            off = 128 * (j - (jmax - 4)) if diag else 0
                            w = TQ - off
                            sc = pp2.tile([128, TQ], f32, tag="sc", bufs=2)
                            nc.tensor.matmul(
                                sc[:, 0:w], kb[:, h, s0:s0 + 128],
                                qb[:, h, t0 + off:t0 + TQ],
                                start=True, stop=True,
                            )
                            ex = exp_pool.tile([128, TQ], bf16, tag="ex")
                            nc.scalar.activation(ex[:, 0:w], sc[:, 0:w], Exp,
                                                 scale=float(SCALE))
                            if diag:
                                exm = exp_pool.tile([128, TQ], bf16, tag="exm",
                                                    bufs=2)
                                nc.vector.tensor_mul(exm[:, 0:w], ex[:, 0:w],
                                                     msk_sb[:, 0:w])
                                ex = exm
                            nc.tensor.matmul(
                                av[:, off:TQ], vb[:, j, h * 128:(h + 1) * 128],
                                ex[:, 0:w],
                                start=(j == 0), stop=(j == jmax - 1),
                            )
                            # softmax denominator: accumulate exp tiles in
                            # bf16 on DVE; one ones-matmul per query tile
                            if j == 0:
                                nc.vector.tensor_copy(exa[:], ex[:])
                            else:
                                nc.vector.tensor_add(exa[:, off:TQ],
                                                     exa[:, off:TQ],
                                                     ex[:, 0:w])
                        cs = pp2.tile([128, TQ], f32, tag="cs", bufs=1)
                        nc.tensor.matmul(cs[:], ones_col[:], exa[:],
                                         start=True, stop=True)
                        # 1/colsum: approx reciprocal (~18 bits, 1 DVE op)
                        rec = ap.tile([128, TQ], f32, tag="rec", bufs=2)
                        nc.vector.reciprocal_approx_fast(rec[:], cs[:])
                        ot = ap.tile([128, TQ], bf16, tag="ot", bufs=3)
                        nc.vector.tensor_mul(ot[:], av[:], rec[:])
                        # queries [t0, t0+512) of batch b go to dest cores
                        # 2tq (first 256) and 2tq+1 (second 256)
                        nc.sync.dma_start(a2a_in[b][h][2 * tq], ot[:, 0:RPB])
                        nc.sync.dma_start(a2a_in[b][h][2 * tq + 1],
                                          ot[:, RPB:2 * RPB])
                        ots.append(ot)
                    return ots

                def a2a(b, h):
                    nc.gpsimd.collective_compute(
                        "AllToAll", mybir.AluOpType.bypass,
                        replica_groups=[list(range(NCORES))],
                        ins=[a2a_in[b][h][:].opt()],
                        outs=[a2a_out[b][h][:].opt()],
                    )

                def load_at(b, dep=None):
                    """Load the resharded attention rows of batch b into SBUF.
                    dep (a tile) token-binds the loads so the scheduler cannot
                    place dependent matmuls where a collective is still in
                    flight on hardware."""
                    at_sb = atp.tile([128, KT, RPB], bf16, tag="at",
                                     name=f"at{b}")
                    if dep is not None:
                        nc.vector.tensor_copy(at_sb[0:1, :, 0:1],
                                              dep[0:1, 0:KT])
                    for h in range(HPC):
                        nc.scalar.dma_start(
                            at_sb[:, h:KT:2, :],
                            a2a_out[b][h].rearrange("i p r -> p i r"))
                    return at_sb

                def op_group(b, at_sb, wo_f, f, tt, store_eng):
                    """One out-projection group: rows [tt*128,(tt+1)*128) of
                    this core's batch-b shard x output cols [f*TQ,(f+1)*TQ)."""
                    ops = pp1.tile([128, TQ], f32, tag="qk", bufs=2)
                    for ki in range(KT):
                        nc.tensor.matmul(
                            ops[:],
                            at_sb[:, ki, tt * 128:(tt + 1) * 128],
                            wo_f[:, ki, :],
                            start=(ki == 0), stop=(ki == KT - 1),
                        )
                    res = resp.tile([128, TQ], bf16, tag="res")
                    nc.vector.tensor_copy(res[:], ops[:])
                    store_eng.dma_start(
                        out_d[b * RPB + tt * 128:b * RPB + (tt + 1) * 128,
                              f * TQ:(f + 1) * TQ],
                        res[:])
                    return res

                # ---- main pipeline ----
                wo_tiles = {}
                for b in range(B):
                    qb, kb, vb = p1(b)
                    ot0 = attention(b, 0, qb, kb, vb)
                    a2a(b, 0)
                    if b == B - 1:
                        # out_proj(b2) interleaves with batch-3 attention:
                        # TensorE filler for the ACT-bound exp chain, and the
                        # only pre-tail out-proj.  at loads token-bound to
                        # mid-attention so its matmuls can't stall the queue.
                        at2 = load_at(B - 2, dep=ot0[1])
                        for f in range(DIM // TQ):
                            wo_f = wop.tile([128, KT, TQ], bf16, tag="wo",
                                            bufs=2)
                            nc.scalar.dma_start(
                                wo_f[:], wo_d[f].rearrange("k p t -> p k t"))
                            wo_tiles[f] = wo_f
                            for tt in range(RPB // 128):
                                op_group(B - 2, at2, wo_f, f, tt, nc.gpsimd)
                    ot1 = attention(b, 1, qb, kb, vb)
                    a2a(b, 1)

                # ---- tail: deferred out-projections cover the last two
                # collectives (~100us of matmul vs ~60us of collective) ----
                at_tiles = {0: load_at(0), 1: load_at(1)}
                last_res = None
                for f in range(DIM // TQ):
                    wo_f = wop.tile([128, KT, TQ], bf16, tag="wo", bufs=2)
                    nc.scalar.dma_start(wo_f[:],
                                        wo_d[f].rearrange("k p t -> p k t"))
                    wo_tiles[f] = wo_f
                    for b2 in (0, 1):
                        for tt in range(RPB // 128):
                            eng = nc.sync if (b2 + tt) % 2 == 0 else nc.gpsimd
                            last_res = op_group(b2, at_tiles[b2], wo_f, f, tt,
                                                eng)
                # batch 3: wo f2/f3 still resident from phase 1; f0/f1
                # re-stream behind them.  at3 token-bound to the last phase-1
                # result so its matmuls strictly follow the cover work.
                at3 = load_at(3, dep=last_res)
                for f in (2, 3, 0, 1):
                    if f >= 2:
                        wo_f = wo_tiles[f]
                    else:
                        wo_f = wop.tile([128, KT, TQ], bf16, tag="wo", bufs=2)
                        nc.scalar.dma_start(
                            wo_f[:], wo_d[f].rearrange("k p t -> p k t"))
                    for tt in range(RPB // 128):
                        eng = nc.sync if tt % 2 == 0 else nc.gpsimd
                        op_group(3, at3, wo_f, f, tt, eng)

    nc.compile()
    return nc


def _prep_inputs(x, Wq, Wk, Wv, Wo, causal):
    bf16 = ml_dtypes.bfloat16
    xT = np.ascontiguousarray(x.reshape(BS, DIM).T).astype(bf16)  # [dim, BS]
    # pre-tile: [block n, ktile, 128, 512]
    xTt = np.ascontiguousarray(
        xT.reshape(KT, 128, B * NB, TQ).transpose(2, 0, 1, 3))
    woT = np.ascontiguousarray(Wo.T).astype(bf16)                 # [e, f]
    woTt = np.ascontiguousarray(
        woT.reshape(KT, 128, DIM // TQ, TQ).transpose(2, 0, 1, 3))

    # RoPE tables in [d, pos] layout, tiled over batches; sin pre-signed for
    # rotate_half (rows 0:64 multiply the shifted-up half, hence negative).
    inv_freq = 1.0 / (10000.0 ** (np.arange(0, D, 2, dtype=np.float64) / D))
    t = np.arange(S, dtype=np.float64)
    freqs = np.outer(t, inv_freq)                      # [S, 64]
    emb = np.concatenate([freqs, freqs], axis=-1)      # [S, D]
    cosT = np.tile(np.cos(emb).T.astype(np.float32), (1, B)).astype(bf16)
    sinN = np.sin(emb).T.astype(np.float32)
    sinN[0:64] *= -1.0
    sinT = np.tile(sinN, (1, B)).astype(bf16)

    # single diagonal mask tile (t_local >= s_local)
    ii = np.arange(128)[:, None]
    jj = np.arange(TQ)[None, :]
    masks = (jj >= ii).astype(bf16)

    in_maps = []
    for c in range(NCORES):
        e0, e1 = c * E, (c + 1) * E
        in_maps.append({
            "xT": xTt,
            "wqT": np.ascontiguousarray(Wq[e0:e1].T).astype(bf16).reshape(KT, 128, E),
            "wkT": np.ascontiguousarray(Wk[e0:e1].T).astype(bf16).reshape(KT, 128, E),
            "wvT": np.ascontiguousarray(Wv[e0:e1].T).astype(bf16).reshape(KT, 128, E),
            "woT": woTt,
            "cosT": cosT,
            "sinT": sinT,
            "masks": masks,
        })
    return in_maps


def kernel(x, Wq, Wk, Wv, Wo, mask, _trace=False):
    from concourse.bass_utils import run_bass_kernel_spmd

    m = np.asarray(mask)
    causal = not bool(m.reshape(m.shape[-2], m.shape[-1])[0, -1])

    if causal not in _CACHE:
        _CACHE[causal] = _build(causal)
    nc = _CACHE[causal]

    in_maps = _prep_inputs(np.asarray(x), np.asarray(Wq), np.asarray(Wk),
                           np.asarray(Wv), np.asarray(Wo), causal)
    res = run_bass_kernel_spmd(nc, in_maps, core_ids=list(range(NCORES)),
                               trace=_trace)
    # core c holds rows [c*RPB, (c+1)*RPB) of every batch, b-major
    full = np.empty((B, S, DIM), np.float32)
    for c in range(NCORES):
        rc = res.results[c]["out"].reshape(B, RPB, DIM)
        full[:, c * RPB:(c + 1) * RPB, :] = rc.astype(np.float32)
    if _trace:
        return full, res
    return full
